# revision 2
# baseline (speedup 1.0000x reference)
"""Trainium2 Bass kernel for nn_DA_conv (dynamic depthwise conv + CA attention).

v2 — data-parallel over batch: 16 samples / 8 cores = 2 samples per core.
Partition layout: 128 partitions = (sample s in 0..1) x (channel c in 0..63).

Host supplies feat as fp16 (image buffer) and fp8e4 (two interleaved copies,
the second shifted +2 cols so a DoubleRow Ko pair reads taps (0,-1),(0,+1));
output is written fp16 and upcast on host.  deg stays fp32.

Device pipeline per core:
  prologue: deg mean -> dvec; f/fa via 16 small tile-positioned matmuls that
    land the per-sample values on their own partition halves (no DRAM round
    trips, no partition shifts); kern chain + att chain via block-diagonal
    [16,x] lhsTs; diag fp8/fp16 tap matrices built on DVE.
  blocks 0..5 (taps on PE): per 512-col region 5 fp8 matmuls
    (3 vertical DoubleRow pairs, 1 horizontal DR pair via the shifted copy,
    1 single center tap), ACT Prelu(scale=1/1024) -> act16.
  blocks 6,7 (taps on DVE): tensor_scalar init + 8 scalar_tensor_tensor,
    all reads 4B-aligned via pad1 (left-padded) / pad1b (dense), ACT Prelu.
  1x1 conv: PE matmul with block-diag(W_conv.T) fp16.
  epilogue V (blocks 0-2): DVE attfb = feat*att + bias (one tensor_scalar),
    then tensor_tensor add with the conv psum -> fp16 out.
  epilogue P (blocks 3-7): PE diag(att) matmul into the conv psum,
    ACT Identity(psum + bias) -> fp16 out.

kernel(**inputs) takes FULL numpy inputs, returns FULL [16,64,128,128] f32.
"""
import numpy as np
from contextlib import ExitStack

import concourse.bass as bass
import concourse.tile as tile
from concourse import bacc, mybir
from concourse.bass_utils import run_bass_kernel_spmd

F8 = mybir.dt.float8e4
F16 = mybir.dt.float16
BF16 = mybir.dt.bfloat16
F32 = mybir.dt.float32
AF = mybir.ActivationFunctionType
OP = mybir.AluOpType
DR = mybir.MatmulPerfMode.DoubleRow

N_CORES = 8
B, C, H, W = 16, 64, 128, 128
BC = B // N_CORES          # 2 samples per core
P = BC * C                 # 128 partitions
HW = H * W                 # 16384
DEG, RED = 512, 8
K = 3
WP = W + 2                 # fp16 padded row stride (130)
# fp8 row layout: [pad8 row (144 B) | pad8b row (136 B)] -> stride 280.
# pad8  col c at offset c      (c in 0..129, left/right zero pads)
# pad8b col c at offset 144+c  (pad8b[c] = x[c+1], so the DR Ko pair
#                               (pad8[w-1+1..], pad8b[...]) = x[w-1], x[w+1])
# Ko steps: horizontal pair 144, vertical pair 2*280=560 — both %16==0.
W8A = 144
W8S = 280                  # full fp8 row stride
KSCALE = 1024.0            # fp8 tap weights are kern*1024 (e4m3 range);
                           # undone exactly by Prelu scale=1/1024
BLK = 2048                 # block cols (16 image rows)
NBLK = HW // BLK           # 8
RPB = BLK // W             # rows per block = 16
NPE = 6                    # blocks 0..NPE-1 taps on PE; the rest on DVE
V_BLOCKS = (0, 1, 2)       # DVE epilogue (attfb + tt); others ACT epilogue
PAD1B_BLOCKS = (6, 7, 5, 0, 1, 2)  # dense f16 copy: DVE taps need 5,6,7
                                   # (5 = row-95 halo), V blocks need 0,1,2

_CACHE = {}


def _ti(di, dj):
    return (di + 1) * 3 + (dj + 1)


def _build():
    nc = bacc.Bacc("TRN2", target_bir_lowering=False, debug=False,
                   num_devices=N_CORES)
    feat16 = nc.declare_dram_parameter("feat16", [BC, C, H, W], F16,
                                       isOutput=False)
    feat8 = nc.declare_dram_parameter("feat8", [BC, C, H, W], F8,
                                      isOutput=False)
    deg = nc.declare_dram_parameter("deg", [BC, DEG, 64], F32, isOutput=False)
    wsz = nc.declare_dram_parameter("wsz", [128, 4 * C], BF16, isOutput=False)
    wac = nc.declare_dram_parameter("wac", [128, 4 * C], BF16, isOutput=False)
    wk1blk = nc.declare_dram_parameter("wk1blk", [128, 16], BF16,
                                       isOutput=False)
    wdu1blk = nc.declare_dram_parameter("wdu1blk", [128, 16], BF16,
                                        isOutput=False)
    wk2b = nc.declare_dram_parameter("wk2b", [16, 9 * 128], BF16,
                                     isOutput=False)
    wdu2b = nc.declare_dram_parameter("wdu2b", [16, 128], BF16,
                                      isOutput=False)
    w2blk = nc.declare_dram_parameter("w2blk", [P, P], F16, isOutput=False)
    bias_p = nc.declare_dram_parameter("bias_p", [P, 1], F32, isOutput=False)
    eye16 = nc.declare_dram_parameter("eye16", [P, P], F16, isOutput=False)
    eye8 = nc.declare_dram_parameter("eye8", [P, P], F8, isOutput=False)
    out16 = nc.declare_dram_parameter("out16", [BC, C, H, W], F16,
                                      isOutput=True)

    feat16v = feat16.ap().rearrange("s c h w -> (s c) h w")
    feat8v = feat8.ap().rearrange("s c h w -> (s c) h w")
    out16v = out16.ap().rearrange("s c h w -> (s c) (h w)")

    with tile.TileContext(nc) as tc:
        with ExitStack() as ctx:
            const = ctx.enter_context(tc.tile_pool(name="const", bufs=1))
            padp = ctx.enter_context(tc.tile_pool(name="padp", bufs=1))

            pad1 = padp.tile([P, H * WP], F16)
            pad1v = pad1[:].rearrange("p (h w) -> p h w", w=WP)
            pad1b = padp.tile([P, H * W], F16)
            pad1bv = pad1b[:].rearrange("p (h w) -> p h w", w=W)
            pad8i = padp.tile([P, (H + 2) * W8S], F8)
            pad8iv = pad8i[:].rearrange("p (h u) -> p h u", u=W8S)

            def pad8_ap(flat_off, dims):
                base = pad8i[:]
                return bass.AP(base.tensor, base.offset + flat_off,
                               [list(base.ap[0])] + [list(d) for d in dims])

            # ---- const DMAs ----
            w2blk_sb = const.tile([P, P], F16)
            nc.sync.dma_start(w2blk_sb[:], w2blk.ap())
            bias_sb = const.tile([P, 1], F32)
            nc.sync.dma_start(bias_sb[:], bias_p.ap())
            eye16_sb = const.tile([P, P], F16)
            nc.sync.dma_start(eye16_sb[:], eye16.ap())
            eye8_sb = const.tile([P, P], F8)
            nc.sync.dma_start(eye8_sb[:], eye8.ap())
            wsz_sb = const.tile([128, 4 * C], BF16)
            nc.sync.dma_start(wsz_sb[:], wsz.ap())
            wac_sb = const.tile([128, 4 * C], BF16)
            nc.sync.dma_start(wac_sb[:], wac.ap())
            wk1blk_sb = const.tile([128, 16], BF16)
            nc.sync.dma_start(wk1blk_sb[:], wk1blk.ap())
            wdu1blk_sb = const.tile([128, 16], BF16)
            nc.sync.dma_start(wdu1blk_sb[:], wdu1blk.ap())
            wk2b_sb = const.tile([16, 9 * 128], BF16)
            nc.sync.dma_start(wk2b_sb[:], wk2b.ap())
            wdu2b_sb = const.tile([16, 128], BF16)
            nc.sync.dma_start(wdu2b_sb[:], wdu2b.ap())

            kern_p = const.tile([P, 9], F32)
            kern1k = const.tile([P, 9], F32)
            att_p = const.tile([P, 1], F32)
            drlhs8 = const.tile([P, 3 * 2 * P], F8)   # vertical DR pairs
            drh8 = const.tile([P, 2 * P], F8)         # horizontal DR pair
            diag00 = const.tile([P, P], F8)           # center tap single
            attd16 = const.tile([P, P], F16)

            # ---- zero pads ----
            nc.vector.memset(pad1v[:, :, 0:1], 0.0)
            nc.vector.memset(pad1v[:, :, WP - 1:WP], 0.0)
            nc.vector.memset(pad8iv[:, :, 0:1], 0.0)          # pad8 col -1
            nc.vector.memset(pad8iv[:, :, W + 1:W + 2], 0.0)  # pad8 col 128
            nc.vector.memset(pad8iv[:, 0, 0:W + 2], 0.0)      # pad8 row -1
            nc.vector.memset(pad8iv[:, :, W8A + W - 1:W8A + W], 0.0)

            # ---- input DMAs (deg first; DVE-tap blocks' f16 early;
            #      then per-PE-block f16+f8 so block 0 is ready soonest) ----
            dgp = ctx.enter_context(tc.tile_pool(name="dgp", bufs=1))
            dg = dgp.tile([128, 2 * 256], F32)
            for s in range(BC):
                nc.sync.dma_start(
                    dg[:, s * 256:(s + 1) * 256].rearrange(
                        "p (t f) -> p t f", t=4),
                    deg.ap()[s].rearrange("(t p) f -> p t f", p=128))

            def dma_feat16(b):
                r0 = b * RPB
                nc.sync.dma_start(pad1v[:, r0:r0 + RPB, 1:1 + W],
                                  feat16v[:, r0:r0 + RPB, :])

            def dma_feat8(b):
                r0 = b * RPB
                nc.sync.dma_start(pad8iv[:, r0 + 1:r0 + RPB + 1, 1:1 + W],
                                  feat8v[:, r0:r0 + RPB, :])
                nc.sync.dma_start(
                    pad8iv[:, r0 + 1:r0 + RPB + 1, W8A:W8A + W - 1],
                    feat8v[:, r0:r0 + RPB, 1:W])

            dma_feat16(NPE)
            dma_feat16(NPE + 1)
            for b in range(NPE):
                dma_feat16(b)
                dma_feat8(b)
            # halo: block NPE-1's di=+1 DR reads image row NPE*RPB
            nc.sync.dma_start(
                pad8iv[:, NPE * RPB + 1:NPE * RPB + 2, 1:1 + W],
                feat8v[:, NPE * RPB:NPE * RPB + 1, :])

            # ---- dense f16 copies (GpSimd; frees DVE for tap chains) ----
            for b in PAD1B_BLOCKS:
                r0 = b * RPB
                nc.gpsimd.tensor_copy(pad1bv[:, r0:r0 + RPB, :],
                                      pad1v[:, r0:r0 + RPB, 1:1 + W])

            # ---- prologue: dvec -> f/fa -> kern + att, all on-device ----
            with ExitStack() as pctx:
                pro = pctx.enter_context(tc.tile_pool(name="pro", bufs=1))
                pps = pctx.enter_context(
                    tc.tile_pool(name="pps", bufs=2, space="PSUM"))

                dv = pro.tile([128, 8], F32)
                nc.vector.tensor_reduce(
                    dv[:], dg[:].rearrange("p (s t f) -> p s t f", s=2, f=64),
                    axis=mybir.AxisListType.X, op=OP.add)
                dv16 = pro.tile([128, 8], BF16)
                nc.vector.tensor_scalar_mul(dv16[:], dv[:], 1.0 / 64.0)

                # f/fa stacked s-major on partitions: fpsum[s*64+o, 0]=f[s,o],
                # [.,1]=fa[s,o] via m=64 matmuls at tile col offset 64*s
                fpsum = pps.tile([128, 2], F32)
                for col, wsb in ((0, wsz_sb), (1, wac_sb)):
                    for s in range(BC):
                        for t in range(4):
                            nc.tensor.matmul(
                                fpsum[64 * s:64 * (s + 1), col:col + 1],
                                wsb[:, t * C:(t + 1) * C],
                                dv16[:, s * 4 + t:s * 4 + t + 1],
                                start=(t == 0), stop=(t == 3))
                fcat = pro.tile([128, 2], BF16)
                nc.scalar.activation(fcat[:], fpsum[:], AF.Copy)

                # h1/h2 stacked [16,1] each via block-diag [128,16] lhsT
                ph12 = pps.tile([16, 2], F32)
                nc.tensor.matmul(ph12[:, 0:1], wk1blk_sb[:], fcat[:, 0:1],
                                 start=True, stop=True)
                nc.tensor.matmul(ph12[:, 1:2], wdu1blk_sb[:], fcat[:, 1:2],
                                 start=True, stop=True)
                h12l = pro.tile([16, 2], BF16)
                nc.scalar.activation(h12l[:], ph12[:], AF.Prelu, alpha=0.1)

                # kern_p[sc, t] + att via block-diag [16,128] lhsTs
                kern_ps = pps.tile([128, 16], F32)
                for t in range(9):
                    nc.tensor.matmul(kern_ps[:, t:t + 1],
                                     wk2b_sb[:, t * 128:(t + 1) * 128],
                                     h12l[:, 0:1], start=True, stop=True)
                nc.tensor.matmul(kern_ps[:, 9:10], wdu2b_sb[:],
                                 h12l[:, 1:2], start=True, stop=True)
                nc.scalar.activation(kern_p[:], kern_ps[:, 0:9], AF.Copy)
                nc.scalar.activation(att_p[:], kern_ps[:, 9:10], AF.Sigmoid)

            nc.vector.tensor_scalar_mul(kern1k[:], kern_p[:], KSCALE)
            for j, dj in enumerate((-1, 0, 1)):
                tlo, thi = _ti(-1, dj), _ti(1, dj)
                nc.vector.tensor_scalar(
                    drlhs8[:, (2 * j) * P:(2 * j + 1) * P], eye8_sb[:],
                    kern1k[:, tlo:tlo + 1], None, op0=OP.mult)
                nc.vector.tensor_scalar(
                    drlhs8[:, (2 * j + 1) * P:(2 * j + 2) * P], eye8_sb[:],
                    kern1k[:, thi:thi + 1], None, op0=OP.mult)
            tl, tr = _ti(0, -1), _ti(0, 1)
            nc.vector.tensor_scalar(drh8[:, 0:P], eye8_sb[:],
                                    kern1k[:, tl:tl + 1], None, op0=OP.mult)
            nc.vector.tensor_scalar(drh8[:, P:2 * P], eye8_sb[:],
                                    kern1k[:, tr:tr + 1], None, op0=OP.mult)
            t0 = _ti(0, 0)
            nc.vector.tensor_scalar(diag00[:], eye8_sb[:],
                                    kern1k[:, t0:t0 + 1], None, op0=OP.mult)
            nc.vector.tensor_scalar(attd16[:], eye16_sb[:], att_p[:],
                                    None, op0=OP.mult)

            # ---- main loop pools ----
            accp = ctx.enter_context(tc.tile_pool(name="accp", bufs=2))
            actp = ctx.enter_context(tc.tile_pool(name="actp", bufs=6))
            actbp = ctx.enter_context(tc.tile_pool(name="actb", bufs=2))
            atfp = ctx.enter_context(tc.tile_pool(name="atfp", bufs=2))
            outp = ctx.enter_context(tc.tile_pool(name="outp", bufs=3))
            pdwp = ctx.enter_context(
                tc.tile_pool(name="pdw", bufs=2, space="PSUM"))
            pcvp = ctx.enter_context(
                tc.tile_pool(name="pcv", bufs=2, space="PSUM"))

            accs = {}   # DVE blocks: pre-lrelu f16 acc
            acts = {}   # PE blocks: [act16 half tiles]

            def emit_dve_taps(b):
                r0 = b * RPB
                r1 = r0 + RPB
                acc = accp.tile([P, BLK], F16, tag="acc")
                accv = acc[:].rearrange("p (r w) -> p r w", w=W)
                ti = _ti(0, -1)
                nc.vector.tensor_scalar(
                    accv[:], pad1v[:, r0:r1, 0:W],
                    kern_p[:, ti:ti + 1], None, op0=OP.mult)
                for di in (-1, 0, 1):
                    for dj in (-1, 0, 1):
                        if (di, dj) == (0, -1):
                            continue
                        a0, a1 = r0 + di, r1 + di
                        s0, s1 = max(a0, 0), min(a1, H)
                        o0 = s0 - a0
                        dst = accv[:, o0:o0 + (s1 - s0), :]
                        if dj == 0:
                            src = pad1bv[:, s0:s1, :]
                        else:
                            src = pad1v[:, s0:s1, 1 + dj:1 + dj + W]
                        ti = _ti(di, dj)
                        nc.vector.scalar_tensor_tensor(
                            dst, src, kern_p[:, ti:ti + 1], dst,
                            op0=OP.mult, op1=OP.add)
                accs[b] = acc

            def emit_pe_taps(b):
                r0 = b * RPB
                halves = []
                for half in range(2):
                    pdw = pdwp.tile([P, 1024], F32)
                    pdwv = pdw[:].rearrange("p (r w) -> p r w", w=W)
                    for q in range(2):
                        c0 = r0 + half * 8 + q * 4
                        dst = pdwv[:, q * 4:q * 4 + 4, :]
                        for j, dj in enumerate((-1, 0, 1)):
                            nc.tensor.matmul(
                                dst,
                                drlhs8[:, 2 * j * P:2 * (j + 1) * P]
                                .rearrange("p (a m) -> p a m", a=2),
                                pad8_ap(c0 * W8S + 1 + dj,
                                        [[2 * W8S, 2], [W8S, 4], [1, W]]),
                                start=(j == 0), stop=False, perf_mode=DR)
                        nc.tensor.matmul(
                            dst, drh8[:].rearrange("p (a m) -> p a m", a=2),
                            pad8_ap((c0 + 1) * W8S + 0,
                                    [[W8A, 2], [W8S, 4], [1, W]]),
                            start=False, stop=False, perf_mode=DR)
                        nc.tensor.matmul(
                            dst, diag00[:],
                            pad8_ap((c0 + 1) * W8S + 1, [[W8S, 4], [1, W]]),
                            start=False, stop=True)
                    act16 = actp.tile([P, 1024], F16, tag="act")
                    nc.scalar.activation(act16[:], pdw[:], AF.Prelu,
                                         alpha=0.1, scale=1.0 / KSCALE)
                    halves.append(act16)
                acts[b] = halves

            def emit_conv(b):
                r0 = b * RPB
                if b in accs:
                    act16b = actbp.tile([P, BLK], F16, tag="actb")
                    nc.scalar.activation(act16b[:], accs[b][:], AF.Prelu,
                                         alpha=0.1)
                    halves = [act16b[:, 0:1024], act16b[:, 1024:2048]]
                else:
                    halves = [t[:] for t in acts[b]]
                is_v = b in V_BLOCKS
                if is_v:
                    attfb = atfp.tile([P, BLK], F16, tag="attfb")
                    nc.vector.tensor_scalar(
                        attfb[:], pad1bv[:, r0:r0 + RPB, :],
                        att_p[:], bias_sb[:], op0=OP.mult, op1=OP.add)
                ostage = outp.tile([P, BLK], F16)
                for half in range(2):
                    at = halves[half]
                    pcv = pcvp.tile([P, 1024], F32)
                    for q in range(2):
                        c0 = r0 + half * 8 + q * 4
                        nc.tensor.matmul(
                            pcv[:, q * 512:(q + 1) * 512], w2blk_sb[:],
                            at[:, q * 512:(q + 1) * 512],
                            start=True, stop=is_v)
                        if not is_v:
                            nc.tensor.matmul(
                                pcv[:, q * 512:(q + 1) * 512], attd16[:],
                                pad1v[:, c0:c0 + 4, 1:1 + W],
                                start=False, stop=True)
                    if is_v:
                        nc.vector.tensor_tensor(
                            ostage[:, half * 1024:(half + 1) * 1024], pcv[:],
                            attfb[:, half * 1024:(half + 1) * 1024],
                            op=OP.add)
                    else:
                        nc.scalar.activation(
                            ostage[:, half * 1024:(half + 1) * 1024], pcv[:],
                            AF.Identity, bias=bias_sb[:], scale=1.0)
                nc.sync.dma_start(out16v[:, b * BLK:(b + 1) * BLK], ostage[:])

            # DVE tap chains first (DVE stream), then the PE interleave
            emit_dve_taps(NPE)
            emit_dve_taps(NPE + 1)

            CONV_SEQ = list(range(NBLK))
            ci = 0
            for idx in range(NPE):
                emit_pe_taps(idx)
                if idx >= 1:
                    emit_conv(CONV_SEQ[ci])
                    ci += 1
            while ci < NBLK:
                emit_conv(CONV_SEQ[ci])
                ci += 1

    nc.compile()
    return nc


def _prep_host(inputs):
    np16 = mybir.dt.np(F16)
    np8 = mybir.dt.np(F8)
    npb = mybir.dt.np(BF16)
    W_size = np.asarray(inputs["W_size"], np.float32)
    W_ac = np.asarray(inputs["W_ac"], np.float32)
    W_k1 = np.asarray(inputs["W_k1"], np.float32)
    W_k2 = np.asarray(inputs["W_k2"], np.float32)
    W_conv = np.asarray(inputs["W_conv"], np.float32)
    b_conv = np.asarray(inputs["b_conv"], np.float32)
    W_du1 = np.asarray(inputs["W_du1"], np.float32)
    W_du2 = np.asarray(inputs["W_du2"], np.float32)

    wsz = np.ascontiguousarray(
        W_size.T.reshape(4, 128, C).transpose(1, 0, 2).reshape(128, 4 * C)
    ).astype(npb)
    wac = np.ascontiguousarray(
        W_ac.T.reshape(4, 128, C).transpose(1, 0, 2).reshape(128, 4 * C)
    ).astype(npb)

    wk1blk = np.zeros((128, 16), np.float32)
    wdu1blk = np.zeros((128, 16), np.float32)
    for s in range(BC):
        wk1blk[s * C:(s + 1) * C, s * RED:(s + 1) * RED] = W_k1.T
        wdu1blk[s * C:(s + 1) * C, s * RED:(s + 1) * RED] = W_du1.T

    tmp = W_k2.reshape(C, 9, RED)          # [c, t, r]
    wk2b = np.zeros((16, 9, 128), np.float32)
    wdu2b = np.zeros((16, 128), np.float32)
    for s in range(BC):
        wk2b[s * RED:(s + 1) * RED, :, s * C:(s + 1) * C] = \
            tmp.transpose(2, 1, 0)
        wdu2b[s * RED:(s + 1) * RED, s * C:(s + 1) * C] = W_du2.T

    w2blk = np.zeros((P, P), np16)
    wct = W_conv.T.astype(np16)
    w2blk[0:C, 0:C] = wct
    w2blk[C:2 * C, C:2 * C] = wct
    bias = np.tile(b_conv, BC).reshape(P, 1)
    return dict(
        wsz=wsz, wac=wac,
        wk1blk=wk1blk.astype(npb), wdu1blk=wdu1blk.astype(npb),
        wk2b=np.ascontiguousarray(wk2b.reshape(16, 9 * 128)).astype(npb),
        wdu2b=wdu2b.astype(npb),
        w2blk=w2blk, bias_p=np.ascontiguousarray(bias.astype(np.float32)),
        eye16=np.eye(P, dtype=np16), eye8=np.eye(P).astype(np8))


def _make_in_maps(inputs):
    shared = _prep_host(inputs)
    np16 = mybir.dt.np(F16)
    np8 = mybir.dt.np(F8)
    feat = np.asarray(inputs["feat"], np.float32)
    f16 = np.ascontiguousarray(feat.astype(np16))
    f8 = np.ascontiguousarray(feat.astype(np8))
    degf = np.ascontiguousarray(
        np.asarray(inputs["deg"], np.float32).reshape(B, DEG, 64))
    in_maps = []
    for i in range(N_CORES):
        m = dict(shared)
        m["feat16"] = f16[i * BC:(i + 1) * BC]
        m["feat8"] = f8[i * BC:(i + 1) * BC]
        m["deg"] = degf[i * BC:(i + 1) * BC]
        in_maps.append(m)
    return in_maps


def kernel(**inputs):
    if "nc" not in _CACHE:
        _CACHE["nc"] = _build()
    nc = _CACHE["nc"]

    in_maps = _make_in_maps(inputs)
    res = None
    for attempt in range(3):
        try:
            res = run_bass_kernel_spmd(nc, in_maps, core_ids=list(range(N_CORES)))
            break
        except Exception:
            # first execution of a freshly compiled NEFF occasionally fails
            # with a transient device error; a retry succeeds
            if attempt == 2:
                raise
            import time
            time.sleep(5)
    out = np.concatenate(
        [np.asarray(res.results[i]["out16"]) for i in range(N_CORES)], axis=0)
    return out.astype(np.float32)


# revision 3
# speedup vs baseline: 1.4820x; 1.4820x over previous
"""Trainium2 Bass kernel for nn_DA_conv (dynamic depthwise conv + CA attention).

v3 — data-parallel over batch: 16 samples / 8 cores = 2 samples per core.
Partition layout: 128 partitions = (sample s in 0..1) x (channel c in 0..63).

Host supplies feat three ways, all DMA'd as fully dense per-partition runs:
  feat16   fp16 dense image  -> pad1b (att residual, DVE dj=0 taps, attfb)
  f16cd    fp16 rows 95..127 shifted left/right one col (zero edge cols)
           -> DVE dj=+-1 taps, all reads flat so DVE hits 2x/4x modes
  feat8p   fp8e4, host-prepadded rows [-1..128] x [pad(144B)|shifted(136B)]
           layout (stride 280) with zero cols baked in -> PE tap matmuls
Output fp16, upcast on host.  deg fp32.

Device pipeline per core:
  prologue: deg mean -> dvec; f/fa via 16 tile-positioned m=64 matmuls that
    land per-sample values on their own partition halves (no DRAM round
    trips); kern + att chains via block-diagonal [16,x] lhsTs; fp8/fp16 tap
    diagonals built on DVE.
  blocks 0..5 (taps on PE): per 512-col region 5 fp8 matmuls
    (3 vertical DoubleRow pairs Ko=560B, 1 horizontal DR pair Ko=144B via
    the shifted copy, 1 single center tap), ACT Prelu(1/1024) -> act16.
  blocks 6,7 (taps on DVE): tensor_scalar init + 8 scalar_tensor_tensor,
    ACT Prelu.
  1x1 conv: PE matmul with block-diag(W_conv.T) fp16.
  epilogue V (blocks 0-2): DVE attf = feat*att (4x), then
    stt((attf + bias) + conv_psum) -> fp16 out.
  epilogue P (blocks 3-7): PE diag(att) matmul into the conv psum,
    ACT Identity(psum + bias) -> fp16 out.
DMA: consts on the scalar HW-DGE queue; deg/feat/out on the sync queue,
ordered so block b's fp8 lands just before its taps.

kernel(**inputs) takes FULL numpy inputs, returns FULL [16,64,128,128] f32.
"""
import numpy as np
from contextlib import ExitStack

import concourse.bass as bass
import concourse.tile as tile
from concourse import bacc, mybir
from concourse.bass_utils import run_bass_kernel_spmd

F8 = mybir.dt.float8e4
F16 = mybir.dt.float16
BF16 = mybir.dt.bfloat16
F32 = mybir.dt.float32
AF = mybir.ActivationFunctionType
OP = mybir.AluOpType
DR = mybir.MatmulPerfMode.DoubleRow

N_CORES = 8
B, C, H, W = 16, 64, 128, 128
BC = B // N_CORES          # 2 samples per core
P = BC * C                 # 128 partitions
HW = H * W                 # 16384
DEG, RED = 512, 8
K = 3
W8A = 144                  # fp8 row: [pad8 144B | pad8b 136B] -> stride 280
W8S = 280                  # vertical DR Ko = 2*280 = 560, horizontal = 144
H8 = H + 2                 # fp8 rows: image rows -1..128
KSCALE = 1024.0            # fp8 tap weights are kern*1024 (e4m3 range);
                           # undone exactly by Prelu scale=1/1024
BLK = 2048                 # block cols (16 image rows)
NBLK = HW // BLK           # 8
RPB = BLK // W             # rows per block = 16
NPE = 6                    # blocks 0..NPE-1 taps on PE; 6,7 on DVE
V_BLOCKS = (0, 1, 2)       # DVE epilogue (attf + stt); others ACT epilogue
CD0 = (NPE * RPB) - 1      # first row held in the f16cd buffer (95)
NCD = H - CD0              # 33 rows

_CACHE = {}


def _ti(di, dj):
    return (di + 1) * 3 + (dj + 1)


def _build():
    nc = bacc.Bacc("TRN2", target_bir_lowering=False, debug=False,
                   num_devices=N_CORES)
    feat16 = nc.declare_dram_parameter("feat16", [BC, C, H, W], F16,
                                       isOutput=False)
    f16cd = nc.declare_dram_parameter("f16cd", [BC, C, 2 * NCD * W], F16,
                                      isOutput=False)
    feat8p = nc.declare_dram_parameter("feat8p", [BC, C, H8 * W8S], F8,
                                       isOutput=False)
    deg = nc.declare_dram_parameter("deg", [BC, DEG, 64], F32, isOutput=False)
    cstb = nc.declare_dram_parameter("cstb", [128, 8 * C + 32], BF16,
                                     isOutput=False)
    cstk = nc.declare_dram_parameter("cstk", [16, 10 * 128], BF16,
                                     isOutput=False)
    cstf = nc.declare_dram_parameter("cstf", [P, 2 * P], F16, isOutput=False)
    eye8 = nc.declare_dram_parameter("eye8", [P, P], F8, isOutput=False)
    bias_pp = nc.declare_dram_parameter("bias_p", [P, 1], F32, isOutput=False)
    out16 = nc.declare_dram_parameter("out16", [BC, C, H, W], F16,
                                      isOutput=True)

    feat16v = feat16.ap().rearrange("s c h w -> (s c) (h w)")
    f16cdv = f16cd.ap().rearrange("s c x -> (s c) x")
    feat8pv = feat8p.ap().rearrange("s c x -> (s c) x")
    out16v = out16.ap().rearrange("s c h w -> (s c) (h w)")

    with tile.TileContext(nc) as tc:
        with ExitStack() as ctx:
            const = ctx.enter_context(tc.tile_pool(name="const", bufs=1))
            padp = ctx.enter_context(tc.tile_pool(name="padp", bufs=1))

            pad1b = padp.tile([P, H * W], F16)          # dense image
            pad1bv = pad1b[:].rearrange("p (h w) -> p h w", w=W)
            cd = padp.tile([P, 2 * NCD * W], F16)       # x<<1 | x>>1, r>=95
            cdv = cd[:].rearrange("p (k h w) -> p k h w", k=2, w=W)
            pad8i = padp.tile([P, H8 * W8S], F8)
            pad8if = pad8i[:]

            def pad8_ap(flat_off, dims):
                return bass.AP(pad8if.tensor, pad8if.offset + flat_off,
                               [list(pad8if.ap[0])] + [list(d) for d in dims])

            # ---- const DMAs (scalar HW-DGE queue; sync stays free) ----
            cstb_sb = const.tile([128, 8 * C + 32], BF16)
            nc.scalar.dma_start(cstb_sb[:], cstb.ap())
            wsz_sb = cstb_sb[:, 0:4 * C]
            wac_sb = cstb_sb[:, 4 * C:8 * C]
            wk1blk_sb = cstb_sb[:, 8 * C:8 * C + 16]
            wdu1blk_sb = cstb_sb[:, 8 * C + 16:8 * C + 32]
            cstk_sb = const.tile([16, 10 * 128], BF16)
            nc.scalar.dma_start(cstk_sb[:], cstk.ap())
            wk2b_sb = cstk_sb[:, 0:9 * 128]
            wdu2b_sb = cstk_sb[:, 9 * 128:10 * 128]
            cstf_sb = const.tile([P, 2 * P], F16)
            nc.scalar.dma_start(cstf_sb[:], cstf.ap())
            w2blk_sb = cstf_sb[:, 0:P]
            eye16_sb = cstf_sb[:, P:2 * P]
            eye8_sb = const.tile([P, P], F8)
            nc.scalar.dma_start(eye8_sb[:], eye8.ap())
            bias_sb = const.tile([P, 1], F32)
            nc.scalar.dma_start(bias_sb[:], bias_pp.ap())

            kern_p = const.tile([P, 9], F32)
            kern1k = const.tile([P, 9], F32)
            att_p = const.tile([P, 1], F32)
            drlhs8 = const.tile([P, 3 * 2 * P], F8)   # vertical DR pairs
            drh8 = const.tile([P, 2 * P], F8)         # horizontal DR pair
            diag00 = const.tile([P, P], F8)           # center tap single
            attd16 = const.tile([P, P], F16)

            # ---- input DMAs (sync queue, in order of first use) ----
            dgp = ctx.enter_context(tc.tile_pool(name="dgp", bufs=1))
            dg = dgp.tile([128, 2 * 256], F32)
            for s in range(BC):
                nc.sync.dma_start(
                    dg[:, s * 256:(s + 1) * 256].rearrange(
                        "p (t f) -> p t f", t=4),
                    deg.ap()[s].rearrange("(t p) f -> p t f", p=128))

            def dma_feat16(b):
                nc.sync.dma_start(pad1b[:, b * BLK:(b + 1) * BLK],
                                  feat16v[:, b * BLK:(b + 1) * BLK])

            def dma_feat8(b):
                # rows r0+1 .. r0+17 of pad8i (18 rows for block 0 so the
                # host-zeroed image row -1 lands too); includes next block's
                # first row as halo for the vertical DR pairs
                lo = 0 if b == 0 else b * RPB + 1
                hi = min(b * RPB + RPB + 2, H8)
                nc.sync.dma_start(pad8i[:, lo * W8S:hi * W8S],
                                  feat8pv[:, lo * W8S:hi * W8S])

            dma_feat16(NPE)
            dma_feat16(NPE + 1)
            dma_feat8(0)
            dma_feat16(0)
            nc.sync.dma_start(cd[:], f16cdv[:])
            for b in (1, 2):
                dma_feat8(b)
                dma_feat16(b)
            for b in range(3, NPE):
                dma_feat8(b)
            for b in range(3, NPE):
                dma_feat16(b)

            # ---- prologue: dvec -> f/fa -> kern + att, all on-device ----
            with ExitStack() as pctx:
                pro = pctx.enter_context(tc.tile_pool(name="pro", bufs=1))
                pps = pctx.enter_context(
                    tc.tile_pool(name="pps", bufs=2, space="PSUM"))

                dv = pro.tile([128, 8], F32)
                nc.vector.tensor_reduce(
                    dv[:], dg[:].rearrange("p (s t f) -> p s t f", s=2, f=64),
                    axis=mybir.AxisListType.X, op=OP.add)
                dv16 = pro.tile([128, 8], BF16)
                nc.vector.tensor_scalar_mul(dv16[:], dv[:], 1.0 / 64.0)

                # f/fa stacked s-major on partitions: fpsum[s*64+o, 0]=f[s,o],
                # [.,1]=fa[s,o] via m=64 matmuls at col tile offset 64*s
                fpsum = pps.tile([128, 2], F32)
                for col, wsb in ((0, wsz_sb), (1, wac_sb)):
                    for s in range(BC):
                        for t in range(4):
                            nc.tensor.matmul(
                                fpsum[64 * s:64 * (s + 1), col:col + 1],
                                wsb[:, t * C:(t + 1) * C],
                                dv16[:, s * 4 + t:s * 4 + t + 1],
                                start=(t == 0), stop=(t == 3))
                fcat = pro.tile([128, 2], BF16)
                nc.scalar.activation(fcat[:], fpsum[:], AF.Copy)

                # h1/h2 stacked [16,1] each via block-diag [128,16] lhsT
                ph12 = pps.tile([16, 2], F32)
                nc.tensor.matmul(ph12[:, 0:1], wk1blk_sb, fcat[:, 0:1],
                                 start=True, stop=True)
                nc.tensor.matmul(ph12[:, 1:2], wdu1blk_sb, fcat[:, 1:2],
                                 start=True, stop=True)
                h12l = pro.tile([16, 2], BF16)
                nc.scalar.activation(h12l[:], ph12[:], AF.Prelu, alpha=0.1)

                # kern_p[sc, t] + att via block-diag [16,128] lhsTs
                kern_ps = pps.tile([128, 16], F32)
                for t in range(9):
                    nc.tensor.matmul(kern_ps[:, t:t + 1],
                                     wk2b_sb[:, t * 128:(t + 1) * 128],
                                     h12l[:, 0:1], start=True, stop=True)
                nc.tensor.matmul(kern_ps[:, 9:10], wdu2b_sb,
                                 h12l[:, 1:2], start=True, stop=True)
                nc.scalar.activation(kern_p[:], kern_ps[:, 0:9], AF.Copy)
                nc.scalar.activation(att_p[:], kern_ps[:, 9:10], AF.Sigmoid)

            nc.vector.tensor_scalar_mul(kern1k[:], kern_p[:], KSCALE)
            for j, dj in enumerate((-1, 0, 1)):
                tlo, thi = _ti(-1, dj), _ti(1, dj)
                nc.vector.tensor_scalar(
                    drlhs8[:, (2 * j) * P:(2 * j + 1) * P], eye8_sb[:],
                    kern1k[:, tlo:tlo + 1], None, op0=OP.mult)
                nc.vector.tensor_scalar(
                    drlhs8[:, (2 * j + 1) * P:(2 * j + 2) * P], eye8_sb[:],
                    kern1k[:, thi:thi + 1], None, op0=OP.mult)
            tl, tr = _ti(0, -1), _ti(0, 1)
            nc.vector.tensor_scalar(drh8[:, 0:P], eye8_sb[:],
                                    kern1k[:, tl:tl + 1], None, op0=OP.mult)
            nc.vector.tensor_scalar(drh8[:, P:2 * P], eye8_sb[:],
                                    kern1k[:, tr:tr + 1], None, op0=OP.mult)
            t0 = _ti(0, 0)
            nc.vector.tensor_scalar(diag00[:], eye8_sb[:],
                                    kern1k[:, t0:t0 + 1], None, op0=OP.mult)
            nc.vector.tensor_scalar(attd16[:], eye16_sb, att_p[:],
                                    None, op0=OP.mult)

            # ---- main loop pools ----
            accp = ctx.enter_context(tc.tile_pool(name="accp", bufs=2))
            actp = ctx.enter_context(tc.tile_pool(name="actp", bufs=6))
            actbp = ctx.enter_context(tc.tile_pool(name="actb", bufs=2))
            atfp = ctx.enter_context(tc.tile_pool(name="atfp", bufs=2))
            outp = ctx.enter_context(tc.tile_pool(name="outp", bufs=3))
            pdwp = ctx.enter_context(
                tc.tile_pool(name="pdw", bufs=2, space="PSUM"))
            pcvp = ctx.enter_context(
                tc.tile_pool(name="pcv", bufs=2, space="PSUM"))

            accs = {}   # DVE blocks: pre-lrelu f16 acc
            acts = {}   # PE blocks: [act16 half tiles]

            def dve_src(di, dj, s0, s1):
                # flat f16 view of x[r+di, w+dj] for rows [s0, s1)
                if dj == 0:
                    return pad1bv[:, s0:s1, :]
                k = 0 if dj == 1 else 1
                return cdv[:, k, s0 - CD0:s1 - CD0, :]

            def emit_dve_taps(b):
                r0 = b * RPB
                r1 = r0 + RPB
                acc = accp.tile([P, BLK], F16, tag="acc")
                accv = acc[:].rearrange("p (r w) -> p r w", w=W)
                ti = _ti(0, 0)
                nc.vector.tensor_scalar(
                    accv[:], pad1bv[:, r0:r1, :],
                    kern_p[:, ti:ti + 1], None, op0=OP.mult)
                for dj in (-1, 0, 1):
                    for di in (-1, 0, 1):
                        if (di, dj) == (0, 0):
                            continue
                        a0, a1 = r0 + di, r1 + di
                        s0, s1 = max(a0, 0), min(a1, H)
                        o0 = s0 - a0
                        dst = accv[:, o0:o0 + (s1 - s0), :]
                        ti = _ti(di, dj)
                        nc.vector.scalar_tensor_tensor(
                            dst, dve_src(di, dj, s0, s1),
                            kern_p[:, ti:ti + 1], dst,
                            op0=OP.mult, op1=OP.add)
                accs[b] = acc

            def emit_pe_taps(b):
                r0 = b * RPB
                halves = []
                for half in range(2):
                    pdw = pdwp.tile([P, 1024], F32)
                    pdwv = pdw[:].rearrange("p (r w) -> p r w", w=W)
                    for q in range(2):
                        c0 = r0 + half * 8 + q * 4
                        dst = pdwv[:, q * 4:q * 4 + 4, :]
                        for j, dj in enumerate((-1, 0, 1)):
                            nc.tensor.matmul(
                                dst,
                                drlhs8[:, 2 * j * P:2 * (j + 1) * P]
                                .rearrange("p (a m) -> p a m", a=2),
                                pad8_ap(c0 * W8S + 1 + dj,
                                        [[2 * W8S, 2], [W8S, 4], [1, W]]),
                                start=(j == 0), stop=False, perf_mode=DR)
                        nc.tensor.matmul(
                            dst, drh8[:].rearrange("p (a m) -> p a m", a=2),
                            pad8_ap((c0 + 1) * W8S + 0,
                                    [[W8A, 2], [W8S, 4], [1, W]]),
                            start=False, stop=False, perf_mode=DR)
                        nc.tensor.matmul(
                            dst, diag00[:],
                            pad8_ap((c0 + 1) * W8S + 1, [[W8S, 4], [1, W]]),
                            start=False, stop=True)
                    act16 = actp.tile([P, 1024], F16, tag="act")
                    nc.scalar.activation(act16[:], pdw[:], AF.Prelu,
                                         alpha=0.1, scale=1.0 / KSCALE)
                    halves.append(act16)
                acts[b] = halves

            def emit_conv(b):
                r0 = b * RPB
                if b in accs:
                    act16b = actbp.tile([P, BLK], F16, tag="actb")
                    nc.scalar.activation(act16b[:], accs[b][:], AF.Prelu,
                                         alpha=0.1)
                    halves = [act16b[:, 0:1024], act16b[:, 1024:2048]]
                else:
                    halves = [t[:] for t in acts[b]]
                is_v = b in V_BLOCKS
                if is_v:
                    attf = atfp.tile([P, BLK], F16, tag="attf")
                    nc.vector.tensor_scalar_mul(
                        attf[:], pad1b[:, b * BLK:(b + 1) * BLK], att_p[:])
                ostage = outp.tile([P, BLK], F16)
                for half in range(2):
                    at = halves[half]
                    pcv = pcvp.tile([P, 1024], F32)
                    for q in range(2):
                        c0 = r0 + half * 8 + q * 4
                        nc.tensor.matmul(
                            pcv[:, q * 512:(q + 1) * 512], w2blk_sb,
                            at[:, q * 512:(q + 1) * 512],
                            start=True, stop=is_v)
                        if not is_v:
                            nc.tensor.matmul(
                                pcv[:, q * 512:(q + 1) * 512], attd16[:],
                                pad1bv[:, c0:c0 + 4, :],
                                start=False, stop=True)
                    if is_v:
                        nc.vector.scalar_tensor_tensor(
                            ostage[:, half * 1024:(half + 1) * 1024],
                            attf[:, half * 1024:(half + 1) * 1024],
                            bias_sb[:], pcv[:], op0=OP.add, op1=OP.add)
                    else:
                        nc.scalar.activation(
                            ostage[:, half * 1024:(half + 1) * 1024], pcv[:],
                            AF.Identity, bias=bias_sb[:], scale=1.0)
                nc.sync.dma_start(out16v[:, b * BLK:(b + 1) * BLK], ostage[:])

            # DVE tap chains first (DVE stream), then the PE interleave
            emit_dve_taps(NPE)
            emit_dve_taps(NPE + 1)

            CONV_SEQ = list(range(NBLK))
            ci = 0
            for idx in range(NPE):
                emit_pe_taps(idx)
                if idx >= 1:
                    emit_conv(CONV_SEQ[ci])
                    ci += 1
            while ci < NBLK:
                emit_conv(CONV_SEQ[ci])
                ci += 1

    nc.compile()
    return nc


def _prep_host(inputs):
    np16 = mybir.dt.np(F16)
    npb = mybir.dt.np(BF16)
    W_size = np.asarray(inputs["W_size"], np.float32)
    W_ac = np.asarray(inputs["W_ac"], np.float32)
    W_k1 = np.asarray(inputs["W_k1"], np.float32)
    W_k2 = np.asarray(inputs["W_k2"], np.float32)
    W_conv = np.asarray(inputs["W_conv"], np.float32)
    b_conv = np.asarray(inputs["b_conv"], np.float32)
    W_du1 = np.asarray(inputs["W_du1"], np.float32)
    W_du2 = np.asarray(inputs["W_du2"], np.float32)

    cstb = np.zeros((128, 8 * C + 32), np.float32)
    cstb[:, 0:4 * C] = \
        W_size.T.reshape(4, 128, C).transpose(1, 0, 2).reshape(128, 4 * C)
    cstb[:, 4 * C:8 * C] = \
        W_ac.T.reshape(4, 128, C).transpose(1, 0, 2).reshape(128, 4 * C)
    for s in range(BC):
        cstb[s * C:(s + 1) * C, 8 * C + s * RED:8 * C + (s + 1) * RED] = \
            W_k1.T
        cstb[s * C:(s + 1) * C, 8 * C + 16 + s * RED:
             8 * C + 16 + (s + 1) * RED] = W_du1.T

    tmp = W_k2.reshape(C, 9, RED)          # [c, t, r]
    cstk = np.zeros((16, 10, 128), np.float32)
    for s in range(BC):
        cstk[s * RED:(s + 1) * RED, 0:9, s * C:(s + 1) * C] = \
            tmp.transpose(2, 1, 0)
        cstk[s * RED:(s + 1) * RED, 9, s * C:(s + 1) * C] = W_du2.T

    cstf = np.zeros((P, 2 * P), np.float32)
    cstf[0:C, 0:C] = W_conv.T
    cstf[C:2 * C, C:2 * C] = W_conv.T
    cstf[:, P:2 * P] = np.eye(P)
    bias = np.tile(b_conv, BC).reshape(P, 1)
    return dict(
        cstb=cstb.astype(npb),
        cstk=np.ascontiguousarray(cstk.reshape(16, 10 * 128)).astype(npb),
        cstf=cstf.astype(np16),
        eye8=np.eye(P).astype(mybir.dt.np(F8)),
        bias_p=np.ascontiguousarray(bias.astype(np.float32)))


def _make_in_maps(inputs):
    shared = _prep_host(inputs)
    np16 = mybir.dt.np(F16)
    np8 = mybir.dt.np(F8)
    feat = np.asarray(inputs["feat"], np.float32)
    f16 = np.ascontiguousarray(feat.astype(np16))
    f8 = feat.astype(np8)

    # f16cd: rows CD0..H-1 as [x<<1 | x>>1] with zero edge cols
    f16cdf = np.zeros((B, C, 2, NCD, W), np16)
    f16cdf[:, :, 0, :, 0:W - 1] = f16[:, :, CD0:H, 1:W]     # x[r, w+1]
    f16cdf[:, :, 1, :, 1:W] = f16[:, :, CD0:H, 0:W - 1]     # x[r, w-1]
    f16cdf = f16cdf.reshape(B, C, 2 * NCD * W)

    # feat8p: host-prepadded fp8 rows -1..128, layout [x pad | x<<1] / 280B
    f8p = np.zeros((B, C, H8, W8S), np8)
    f8p[:, :, 1:H + 1, 1:W + 1] = f8                        # pad8: x
    f8p[:, :, 1:H + 1, W8A:W8A + W - 1] = f8[:, :, :, 1:W]  # pad8b: x<<1
    f8p = f8p.reshape(B, C, H8 * W8S)

    degf = np.ascontiguousarray(
        np.asarray(inputs["deg"], np.float32).reshape(B, DEG, 64))
    in_maps = []
    for i in range(N_CORES):
        sl = slice(i * BC, (i + 1) * BC)
        m = dict(shared)
        m["feat16"] = f16[sl]
        m["f16cd"] = np.ascontiguousarray(f16cdf[sl])
        m["feat8p"] = np.ascontiguousarray(f8p[sl])
        m["deg"] = degf[sl]
        in_maps.append(m)
    return in_maps


def kernel(**inputs):
    if "nc" not in _CACHE:
        _CACHE["nc"] = _build()
    nc = _CACHE["nc"]

    in_maps = _make_in_maps(inputs)
    res = None
    for attempt in range(3):
        try:
            res = run_bass_kernel_spmd(nc, in_maps,
                                       core_ids=list(range(N_CORES)))
            break
        except Exception:
            # first execution of a freshly compiled NEFF occasionally fails
            # with a transient device error; a retry succeeds
            if attempt == 2:
                raise
            import time
            time.sleep(5)
    out = np.concatenate(
        [np.asarray(res.results[i]["out16"]) for i in range(N_CORES)], axis=0)
    return out.astype(np.float32)


# revision 4
# speedup vs baseline: 1.6331x; 1.1019x over previous
"""Trainium2 Bass kernel for nn_DA_conv (dynamic depthwise conv + CA attention).

v4 — data-parallel over batch: 16 samples / 8 cores = 2 samples per core.
Partition layout: 128 partitions = (sample s in 0..1) x (channel c in 0..63).

Host supplies feat twice, DMA'd as dense per-partition runs in large chunks
split across both HW-DGE queues (sync + scalar, ~230 GB/s each):
  feat16   fp16 dense image -> pad1b (att residual via DVE)
  feat8p   fp8e4, host-prepadded rows [-1..128] x [pad(144B)|shifted(136B)]
           layout (stride 280B) with zero cols baked in -> PE tap matmuls
Output fp16, upcast on host.  deg fp32 (host pre-transposed, dense).

Device pipeline per core:
  prologue: deg mean -> dvec; f/fa via 16 tile-positioned m=64 matmuls that
    land per-sample values on their own partition halves (no DRAM round
    trips); kern + att chains via block-diagonal [16,x] lhsTs; fp8 tap
    diagonals built on DVE.
  all 8 blocks, taps on PE: per 512-col region 5 fp8 matmuls
    (3 vertical DoubleRow pairs Ko=560B, 1 horizontal DR pair Ko=144B via
    the shifted copy, 1 single center tap), ACT Prelu(1/1024) -> act16.
  1x1 conv: PE matmul with block-diag(W_conv.T) fp16, single accumulation
    group per 512-col region.
  epilogue (all blocks): DVE attf = feat16*att (tensor_scalar, 4x mode),
    then scalar_tensor_tensor (attf + bias) + conv_psum -> fp16 out.

kernel(**inputs) takes FULL numpy inputs, returns FULL [16,64,128,128] f32.
"""
import numpy as np
from contextlib import ExitStack

import concourse.bass as bass
import concourse.tile as tile
from concourse import bacc, mybir
from concourse.bass_utils import run_bass_kernel_spmd

F8 = mybir.dt.float8e4
F16 = mybir.dt.float16
BF16 = mybir.dt.bfloat16
F32 = mybir.dt.float32
AF = mybir.ActivationFunctionType
OP = mybir.AluOpType
DR = mybir.MatmulPerfMode.DoubleRow

N_CORES = 8
B, C, H, W = 16, 64, 128, 128
BC = B // N_CORES          # 2 samples per core
P = BC * C                 # 128 partitions
HW = H * W                 # 16384
DEG, RED = 512, 8
K = 3
W8A = 144                  # fp8 row: [pad8 144B | pad8b 136B] -> stride 280
W8S = 280                  # vertical DR Ko = 2*280 = 560, horizontal = 144
H8 = H + 2                 # fp8 rows: image rows -1..128
KSCALE = 1024.0            # fp8 tap weights are kern*1024 (e4m3 range);
                           # undone exactly by Prelu scale=1/1024
BLK = 2048                 # block cols (16 image rows)
NBLK = HW // BLK           # 8
RPB = BLK // W             # rows per block = 16
# fp8 DMA chunks (pad8i row ranges) sized so block b's taps are fed just in
# time while keeping per-partition runs >= 4.5 KB
F8_CHUNKS = ((0, 18), (18, 50), (50, 82), (82, 114), (114, 130))
F16_CHUNKS = ((0, 1), (1, 2), (2, 4), (4, 6), (6, 8))   # feat16, in blocks

_CACHE = {}


def _ti(di, dj):
    return (di + 1) * 3 + (dj + 1)


def _build():
    nc = bacc.Bacc("TRN2", target_bir_lowering=False, debug=False,
                   num_devices=N_CORES)
    feat16 = nc.declare_dram_parameter("feat16", [BC, C, H, W], F16,
                                       isOutput=False)
    feat8p = nc.declare_dram_parameter("feat8p", [BC, C, H8 * W8S], F8,
                                       isOutput=False)
    degt = nc.declare_dram_parameter("degt", [128, 8 * 64], F32,
                                     isOutput=False)
    cstb = nc.declare_dram_parameter("cstb", [128, 8 * C + 32], BF16,
                                     isOutput=False)
    cstk = nc.declare_dram_parameter("cstk", [16, 10 * 128], BF16,
                                     isOutput=False)
    cstf = nc.declare_dram_parameter("cstf", [P, P], F16, isOutput=False)
    eye8 = nc.declare_dram_parameter("eye8", [P, P], F8, isOutput=False)
    bias_pp = nc.declare_dram_parameter("bias_p", [P, 1], F32, isOutput=False)
    out16 = nc.declare_dram_parameter("out16", [BC, C, H, W], F16,
                                      isOutput=True)

    feat16v = feat16.ap().rearrange("s c h w -> (s c) (h w)")
    feat8pv = feat8p.ap().rearrange("s c x -> (s c) x")
    out16v = out16.ap().rearrange("s c h w -> (s c) (h w)")

    with tile.TileContext(nc) as tc:
        with ExitStack() as ctx:
            const = ctx.enter_context(tc.tile_pool(name="const", bufs=1))
            padp = ctx.enter_context(tc.tile_pool(name="padp", bufs=1))

            pad1b = padp.tile([P, H * W], F16)          # dense fp16 image
            pad1bv = pad1b[:].rearrange("p (h w) -> p h w", w=W)
            pad8i = padp.tile([P, H8 * W8S], F8)
            pad8if = pad8i[:]

            def pad8_ap(flat_off, dims):
                return bass.AP(pad8if.tensor, pad8if.offset + flat_off,
                               [list(pad8if.ap[0])] + [list(d) for d in dims])

            # ---- const DMAs (scalar HW-DGE queue) ----
            cstb_sb = const.tile([128, 8 * C + 32], BF16)
            nc.scalar.dma_start(cstb_sb[:], cstb.ap())
            wsz_sb = cstb_sb[:, 0:4 * C]
            wac_sb = cstb_sb[:, 4 * C:8 * C]
            wk1blk_sb = cstb_sb[:, 8 * C:8 * C + 16]
            wdu1blk_sb = cstb_sb[:, 8 * C + 16:8 * C + 32]
            cstk_sb = const.tile([16, 10 * 128], BF16)
            nc.scalar.dma_start(cstk_sb[:], cstk.ap())
            wk2b_sb = cstk_sb[:, 0:9 * 128]
            wdu2b_sb = cstk_sb[:, 9 * 128:10 * 128]
            w2blk_sb = const.tile([P, P], F16)
            nc.scalar.dma_start(w2blk_sb[:], cstf.ap())
            eye8_sb = const.tile([P, P], F8)
            nc.scalar.dma_start(eye8_sb[:], eye8.ap())
            bias_sb = const.tile([P, 1], F32)
            nc.scalar.dma_start(bias_sb[:], bias_pp.ap())

            kern_p = const.tile([P, 9], F32)
            kern1k = const.tile([P, 9], F32)
            att_p = const.tile([P, 1], F32)
            drlhs8 = const.tile([P, 3 * 2 * P], F8)   # vertical DR pairs
            drh8 = const.tile([P, 2 * P], F8)         # horizontal DR pair
            diag00 = const.tile([P, P], F8)           # center tap single

            # ---- input DMAs ----
            dgp = ctx.enter_context(tc.tile_pool(name="dgp", bufs=1))
            dg = dgp.tile([128, 8 * 64], F32)
            nc.sync.dma_start(dg[:], degt.ap())
            for lo, hi in F8_CHUNKS:
                nc.sync.dma_start(pad8i[:, lo * W8S:hi * W8S],
                                  feat8pv[:, lo * W8S:hi * W8S])
            for lo, hi in F16_CHUNKS:
                nc.scalar.dma_start(pad1b[:, lo * BLK:hi * BLK],
                                    feat16v[:, lo * BLK:hi * BLK])

            # ---- prologue: dvec -> f/fa -> kern + att, all on-device ----
            with ExitStack() as pctx:
                pro = pctx.enter_context(tc.tile_pool(name="pro", bufs=1))
                pps = pctx.enter_context(
                    tc.tile_pool(name="pps", bufs=2, space="PSUM"))

                # dg[dp, (s t f)] = deg[s, t*128+dp, f] (host-transposed)
                dv = pro.tile([128, 8], F32)
                nc.vector.tensor_reduce(
                    dv[:], dg[:].rearrange("p (s t f) -> p s t f", s=2, f=64),
                    axis=mybir.AxisListType.X, op=OP.add)
                dv16 = pro.tile([128, 8], BF16)
                nc.vector.tensor_scalar_mul(dv16[:], dv[:], 1.0 / 64.0)

                # f/fa stacked s-major on partitions: fpsum[s*64+o, 0]=f[s,o],
                # [.,1]=fa[s,o] via m=64 matmuls at col tile offset 64*s
                fpsum = pps.tile([128, 2], F32)
                for col, wsb in ((0, wsz_sb), (1, wac_sb)):
                    for s in range(BC):
                        for t in range(4):
                            nc.tensor.matmul(
                                fpsum[64 * s:64 * (s + 1), col:col + 1],
                                wsb[:, t * C:(t + 1) * C],
                                dv16[:, s * 4 + t:s * 4 + t + 1],
                                start=(t == 0), stop=(t == 3))
                fcat = pro.tile([128, 2], BF16)
                nc.scalar.activation(fcat[:], fpsum[:], AF.Copy)

                # h1/h2 stacked [16,1] each via block-diag [128,16] lhsT
                ph12 = pps.tile([16, 2], F32)
                nc.tensor.matmul(ph12[:, 0:1], wk1blk_sb, fcat[:, 0:1],
                                 start=True, stop=True)
                nc.tensor.matmul(ph12[:, 1:2], wdu1blk_sb, fcat[:, 1:2],
                                 start=True, stop=True)
                h12l = pro.tile([16, 2], BF16)
                nc.scalar.activation(h12l[:], ph12[:], AF.Prelu, alpha=0.1)

                # kern_p[sc, t] + att via block-diag [16,128] lhsTs
                kern_ps = pps.tile([128, 16], F32)
                for t in range(9):
                    nc.tensor.matmul(kern_ps[:, t:t + 1],
                                     wk2b_sb[:, t * 128:(t + 1) * 128],
                                     h12l[:, 0:1], start=True, stop=True)
                nc.tensor.matmul(kern_ps[:, 9:10], wdu2b_sb,
                                 h12l[:, 1:2], start=True, stop=True)
                nc.scalar.activation(kern_p[:], kern_ps[:, 0:9], AF.Copy)
                nc.scalar.activation(att_p[:], kern_ps[:, 9:10], AF.Sigmoid)

            nc.vector.tensor_scalar_mul(kern1k[:], kern_p[:], KSCALE)
            for j, dj in enumerate((-1, 0, 1)):
                tlo, thi = _ti(-1, dj), _ti(1, dj)
                nc.vector.tensor_scalar(
                    drlhs8[:, (2 * j) * P:(2 * j + 1) * P], eye8_sb[:],
                    kern1k[:, tlo:tlo + 1], None, op0=OP.mult)
                nc.vector.tensor_scalar(
                    drlhs8[:, (2 * j + 1) * P:(2 * j + 2) * P], eye8_sb[:],
                    kern1k[:, thi:thi + 1], None, op0=OP.mult)
            tl, tr = _ti(0, -1), _ti(0, 1)
            nc.vector.tensor_scalar(drh8[:, 0:P], eye8_sb[:],
                                    kern1k[:, tl:tl + 1], None, op0=OP.mult)
            nc.vector.tensor_scalar(drh8[:, P:2 * P], eye8_sb[:],
                                    kern1k[:, tr:tr + 1], None, op0=OP.mult)
            t0 = _ti(0, 0)
            nc.vector.tensor_scalar(diag00[:], eye8_sb[:],
                                    kern1k[:, t0:t0 + 1], None, op0=OP.mult)

            # ---- main loop pools ----
            actp = ctx.enter_context(tc.tile_pool(name="actp", bufs=6))
            atfp = ctx.enter_context(tc.tile_pool(name="atfp", bufs=3))
            outp = ctx.enter_context(tc.tile_pool(name="outp", bufs=4))
            pdwp = ctx.enter_context(
                tc.tile_pool(name="pdw", bufs=2, space="PSUM"))
            pcvp = ctx.enter_context(
                tc.tile_pool(name="pcv", bufs=2, space="PSUM"))

            acts = {}   # PE blocks: [act16 half tiles]

            def emit_pe_taps(b):
                r0 = b * RPB
                halves = []
                for half in range(2):
                    pdw = pdwp.tile([P, 1024], F32)
                    pdwv = pdw[:].rearrange("p (r w) -> p r w", w=W)
                    for q in range(2):
                        c0 = r0 + half * 8 + q * 4
                        dst = pdwv[:, q * 4:q * 4 + 4, :]
                        for j, dj in enumerate((-1, 0, 1)):
                            nc.tensor.matmul(
                                dst,
                                drlhs8[:, 2 * j * P:2 * (j + 1) * P]
                                .rearrange("p (a m) -> p a m", a=2),
                                pad8_ap(c0 * W8S + 1 + dj,
                                        [[2 * W8S, 2], [W8S, 4], [1, W]]),
                                start=(j == 0), stop=False, perf_mode=DR)
                        nc.tensor.matmul(
                            dst, drh8[:].rearrange("p (a m) -> p a m", a=2),
                            pad8_ap((c0 + 1) * W8S + 0,
                                    [[W8A, 2], [W8S, 4], [1, W]]),
                            start=False, stop=False, perf_mode=DR)
                        nc.tensor.matmul(
                            dst, diag00[:],
                            pad8_ap((c0 + 1) * W8S + 1, [[W8S, 4], [1, W]]),
                            start=False, stop=True)
                    act16 = actp.tile([P, 1024], F16, tag="act")
                    nc.scalar.activation(act16[:], pdw[:], AF.Prelu,
                                         alpha=0.1, scale=1.0 / KSCALE)
                    halves.append(act16)
                acts[b] = halves

            def emit_conv(b):
                halves = acts.pop(b)
                attf = atfp.tile([P, BLK], F16, tag="attf")
                nc.vector.tensor_scalar_mul(
                    attf[:], pad1b[:, b * BLK:(b + 1) * BLK], att_p[:])
                ostage = outp.tile([P, BLK], F16)
                for half in range(2):
                    at = halves[half]
                    pcv = pcvp.tile([P, 1024], F32)
                    for q in range(2):
                        nc.tensor.matmul(
                            pcv[:, q * 512:(q + 1) * 512], w2blk_sb[:],
                            at[:, q * 512:(q + 1) * 512],
                            start=True, stop=True)
                    nc.vector.scalar_tensor_tensor(
                        ostage[:, half * 1024:(half + 1) * 1024],
                        attf[:, half * 1024:(half + 1) * 1024],
                        bias_sb[:], pcv[:], op0=OP.add, op1=OP.add)
                eng = nc.scalar if b >= 6 else nc.sync
                eng.dma_start(out16v[:, b * BLK:(b + 1) * BLK], ostage[:])

            CONV_SEQ = list(range(NBLK))
            ci = 0
            for idx in range(NBLK):
                emit_pe_taps(idx)
                if idx >= 1:
                    emit_conv(CONV_SEQ[ci])
                    ci += 1
            while ci < NBLK:
                emit_conv(CONV_SEQ[ci])
                ci += 1

    nc.compile()
    return nc


def _prep_host(inputs):
    np16 = mybir.dt.np(F16)
    npb = mybir.dt.np(BF16)
    W_size = np.asarray(inputs["W_size"], np.float32)
    W_ac = np.asarray(inputs["W_ac"], np.float32)
    W_k1 = np.asarray(inputs["W_k1"], np.float32)
    W_k2 = np.asarray(inputs["W_k2"], np.float32)
    W_conv = np.asarray(inputs["W_conv"], np.float32)
    b_conv = np.asarray(inputs["b_conv"], np.float32)
    W_du1 = np.asarray(inputs["W_du1"], np.float32)
    W_du2 = np.asarray(inputs["W_du2"], np.float32)

    cstb = np.zeros((128, 8 * C + 32), np.float32)
    cstb[:, 0:4 * C] = \
        W_size.T.reshape(4, 128, C).transpose(1, 0, 2).reshape(128, 4 * C)
    cstb[:, 4 * C:8 * C] = \
        W_ac.T.reshape(4, 128, C).transpose(1, 0, 2).reshape(128, 4 * C)
    for s in range(BC):
        cstb[s * C:(s + 1) * C, 8 * C + s * RED:8 * C + (s + 1) * RED] = \
            W_k1.T
        cstb[s * C:(s + 1) * C, 8 * C + 16 + s * RED:
             8 * C + 16 + (s + 1) * RED] = W_du1.T

    tmp = W_k2.reshape(C, 9, RED)          # [c, t, r]
    cstk = np.zeros((16, 10, 128), np.float32)
    for s in range(BC):
        cstk[s * RED:(s + 1) * RED, 0:9, s * C:(s + 1) * C] = \
            tmp.transpose(2, 1, 0)
        cstk[s * RED:(s + 1) * RED, 9, s * C:(s + 1) * C] = W_du2.T

    cstf = np.zeros((P, P), np.float32)
    cstf[0:C, 0:C] = W_conv.T
    cstf[C:2 * C, C:2 * C] = W_conv.T
    bias = np.tile(b_conv, BC).reshape(P, 1)
    return dict(
        cstb=cstb.astype(npb),
        cstk=np.ascontiguousarray(cstk.reshape(16, 10 * 128)).astype(npb),
        cstf=cstf.astype(np16),
        eye8=np.eye(P).astype(mybir.dt.np(F8)),
        bias_p=np.ascontiguousarray(bias.astype(np.float32)))


def _make_in_maps(inputs):
    shared = _prep_host(inputs)
    np16 = mybir.dt.np(F16)
    np8 = mybir.dt.np(F8)
    feat = np.asarray(inputs["feat"], np.float32)
    f16 = np.ascontiguousarray(feat.astype(np16))
    f8 = feat.astype(np8)

    # feat8p: host-prepadded fp8 rows -1..128, layout [x pad | x<<1] / 280B
    f8p = np.zeros((B, C, H8, W8S), np8)
    f8p[:, :, 1:H + 1, 1:W + 1] = f8                        # pad8: x
    f8p[:, :, 1:H + 1, W8A:W8A + W - 1] = f8[:, :, :, 1:W]  # pad8b: x<<1
    f8p = f8p.reshape(B, C, H8 * W8S)

    # degt[dp, (s t f)] = deg[s, t*128+dp, f], per core
    degf = np.asarray(inputs["deg"], np.float32).reshape(
        N_CORES, BC, 4, 128, 64)
    degt = np.ascontiguousarray(
        degf.transpose(0, 3, 1, 2, 4).reshape(N_CORES, 128, 8 * 64))
    in_maps = []
    for i in range(N_CORES):
        sl = slice(i * BC, (i + 1) * BC)
        m = dict(shared)
        m["feat16"] = f16[sl]
        m["feat8p"] = np.ascontiguousarray(f8p[sl])
        m["degt"] = degt[i]
        in_maps.append(m)
    return in_maps


def kernel(**inputs):
    if "nc" not in _CACHE:
        _CACHE["nc"] = _build()
    nc = _CACHE["nc"]

    in_maps = _make_in_maps(inputs)
    res = None
    for attempt in range(3):
        try:
            res = run_bass_kernel_spmd(nc, in_maps,
                                       core_ids=list(range(N_CORES)))
            break
        except Exception:
            # first execution of a freshly compiled NEFF occasionally fails
            # with a transient device error; a retry succeeds
            if attempt == 2:
                raise
            import time
            time.sleep(5)
    out = np.concatenate(
        [np.asarray(res.results[i]["out16"]) for i in range(N_CORES)], axis=0)
    return out.astype(np.float32)
